# revision 48
# baseline (speedup 1.0000x reference)
"""Multi-head attention (B=8, S=1024, D=768, H=12) on 8 TRN2 NeuronCores.

Sharding: batch-parallel — each core computes one batch item end-to-end
(weights replicated), so no collectives are needed. Host shards x, runs the
SPMD Bass kernel on cores 0-7, gathers per-core outputs.

Per-core dataflow ([feature, seq] transposed layout, fp16 matmuls, fp32 PSUM):
  qT = (W_q/8)^T x^T          [768, 1024]  partitions 0-63 = head 2m,
  kT = W_k^T x^T              [768, 1024]  64-127 = head 2m+1 per m-tile
  v  = x W_v (+ ones col)     [1024, 768]
  scores: per (m, j-block, k-tile i): TWO K=64 matmuls (head pair) into one
    PSUM tile — auto row-tiling (tile_position (0,0)/(64,0)) runs them
    concurrently in the 64x128-tiled PE array.
  exp: single gap-free ACT op per group covering both heads (ACT is the
    attention-phase bottleneck: (N+352)/1.2ns per op).
  attnV: per head, M=65 (64 v-dims + ones row -> softmax denom), N trimmed
    causally; evacuated as one [65,512] DVE copy, rows fanned out by DMA.
  normalize per m: denom rows -> DRAM, reload as [64,32] (all lanes),
    reciprocal, DRAM-bounce broadcast, one [128,1024] fp16 multiply.
  yT = W_o^T out^T: pass1 (k=0..3) interleaved into m=4 attention stalls,
    pass2 (k=4..5) after the last normalize; fp16 stores.
Host transposes/upcasts yT back to [1024, 768] fp32.
"""

import numpy as np

B, S, D, H, DK = 8, 1024, 768, 12, 64
DT = D // 128        # 6  d-model tiles
ST = S // 128        # 8  seq tiles
NB = S // 512        # 2  512-wide seq blocks
HPM = 2              # heads per 128-row m-tile

_CACHE = {}


def _classify_mask(mask_bool):
    """mask_bool: [S, S] (q, k). Per (j, i) block descriptors for the
    scoresT layout [k, q]: list over j (512-wide q blocks) of lists of
    (i, exp_lo, mul(lo,hi)|None, pat_id|None), plus deduped mask patterns
    (each [128, w] f16, [k, q], stored duplicated side by side so one DVE
    op can mask both heads' copies)."""
    patterns = []
    pat_index = {}
    blocks = []
    for j in range(NB):
        row = []
        for i in range(ST):
            sub = mask_bool[j * 512:(j + 1) * 512, i * 128:(i + 1) * 128].T
            # sub: [k 128, q 512]
            if not sub.any():
                continue
            col_any = sub.any(axis=0)
            col_all = sub.all(axis=0)
            exp_lo = int(np.argmax(col_any))
            assert not col_any[:exp_lo].any()
            mixed = ~col_all
            mixed[:exp_lo] = False
            mul = None
            pat_id = None
            if mixed.any():
                lo = int(np.argmax(mixed))
                hi = int(len(mixed) - np.argmax(mixed[::-1]))
                assert col_all[hi:].all() and col_all[exp_lo:lo].all()
                pat = sub[:, lo:hi].astype(np.float16)
                key = (pat.shape[1], pat.tobytes())
                if key not in pat_index:
                    pat_index[key] = len(patterns)
                    patterns.append(pat)
                mul = (lo, hi)
                pat_id = pat_index[key]
            row.append((i, exp_lo, mul, pat_id))
        blocks.append(row)
    return blocks, patterns


def _build(blocks, patterns, pat_width):
    import concourse.bass as bass
    import concourse.bacc as bacc
    import concourse.mybir as mybir
    import concourse.tile as tile
    from contextlib import ExitStack

    f32 = mybir.dt.float32
    f16 = mybir.dt.float16
    AF = mybir.ActivationFunctionType

    nc = bacc.Bacc("TRN2", target_bir_lowering=False, debug=False)

    xT_d = nc.dram_tensor("xT", [3, 128, 2, S], f16,
                          kind="ExternalInput").ap()   # k-pair chunk major
    wq_d = nc.dram_tensor("wq", [128, DT, DT, 128], f16,
                          kind="ExternalInput").ap()   # [p, m, k, c]
    wk_d = nc.dram_tensor("wk", [128, DT, DT, 128], f16,
                          kind="ExternalInput").ap()
    wv_d = nc.dram_tensor("wv", [128, DT, D], f16, kind="ExternalInput").ap()
    wo_d = nc.dram_tensor("wo", [128, DT, D], f16, kind="ExternalInput").ap()
    bqko_d = nc.dram_tensor("bqko", [128, 3, DT], f32,
                            kind="ExternalInput").ap()
    bv_d = nc.dram_tensor("bv", [128, H, DK], f16,
                          kind="ExternalInput").ap()
    yT_d = nc.dram_tensor("yT", [D, S], f16, kind="ExternalOutput").ap()
    if pat_width:
        mk_d = nc.dram_tensor("masks", [128, pat_width], f16,
                              kind="ExternalInput").ap()

    pat_off = []
    off = 0
    for p in patterns:
        pat_off.append(off)
        off += 2 * p.shape[1]

    with tile.TileContext(nc) as tc, ExitStack() as ctx:
        cpool = ctx.enter_context(tc.tile_pool(name="cpool", bufs=1))
        qT = cpool.tile([128, DT, S], f16)
        kT = cpool.tile([128, DT, S], f16)
        vE = cpool.tile([128, ST, H * 65 + 63], f16)
        ao = [cpool.tile([128, S], f16, name=f"ao{m}") for m in range(DT)]
        ya = [cpool.tile([128, 512], f16, name=f"ya{g}") for g in range(12)]
        bvb = cpool.tile([128, H, DK], f16)
        ones1 = cpool.tile([128, 1], f16)
        bqko = cpool.tile([128, 3, DT], f32)
        bqs = bqko[:, 0]
        bks = bqko[:, 1]
        bos = bqko[:, 2]

        xt = cpool.tile([128, DT, S], f16)
        wvt = cpool.tile([128, DT, D], f16)
        wqt = cpool.tile([128, DT, DT, 128], f16)
        wkt = cpool.tile([128, DT, DT, 128], f16)
        wot = cpool.tile([128, DT, D], f16)

        # staged input loads. x is split across three otherwise-idle DMA
        # queues in parallel (per-queue DMA bw is only ~120 GB/s); the
        # sync queue carries just the weights; small constants ride the
        # scalar queue. Chains keep the SDMA round-robin from starving
        # the early pieces.
        mks = cpool.tile([128, pat_width], f16, name="mks") \
            if pat_width else None
        d_x0 = nc.gpsimd.dma_start(out=xt[:, 0:2], in_=xT_d[0])
        d_wva = nc.gpsimd.dma_start(out=wvt[:, :, 0:512],
                                    in_=wv_d[:, :, 0:512])
        d_wvb = nc.gpsimd.dma_start(out=wvt[:, :, 512:768],
                                    in_=wv_d[:, :, 512:768])
        d_wo = nc.gpsimd.dma_start(out=wot, in_=wo_d)
        gchain = [d_x0, d_wva, d_wvb, d_wo]
        d_x1 = nc.scalar.dma_start(out=xt[:, 2:4], in_=xT_d[1])
        cchain = [d_x1, nc.scalar.dma_start(out=bqko, in_=bqko_d)]
        if pat_width:
            cchain.append(nc.scalar.dma_start(out=mks, in_=mk_d))
        cchain.append(nc.scalar.dma_start(out=bvb, in_=bv_d))
        d_wq0 = nc.sync.dma_start(out=wqt[:, 0], in_=wq_d[:, 0])
        d_x2 = nc.sync.dma_start(out=xt[:, 4:6], in_=xT_d[2])
        d_wk0 = nc.sync.dma_start(out=wkt[:, 0], in_=wk_d[:, 0])
        schain = [d_wq0, d_x2, d_wk0,
                  nc.sync.dma_start(out=wqt[:, 1:6], in_=wq_d[:, 1:6]),
                  nc.sync.dma_start(out=wkt[:, 1:6], in_=wk_d[:, 1:6])]
        for chain in (gchain, schain, cchain):
            for a, b in zip(chain[1:], chain):
                tile.add_dep_helper(a.ins, b.ins, reason="stage inputs")

        # ones columns for the softmax-denominator rows; zero the tail so
        # the last heads' 128-col lhsT windows read defined data
        for st in range(ST):
            ve_h = vE[:, st, 0:H * 65].rearrange("p (h e) -> p h e", e=65)
            nc.vector.memset(ve_h[:, :, DK:DK + 1], 1.0)
        nc.vector.memset(vE[:, :, H * 65:], 0.0)
        nc.vector.memset(ones1, 1.0)

        ps_pj = ctx.enter_context(
            tc.tile_pool(name="ps_pj", bufs=2, space="PSUM"))
        ps_s = ctx.enter_context(
            tc.tile_pool(name="ps_s", bufs=2, space="PSUM"))
        ps_o = ctx.enter_context(
            tc.tile_pool(name="ps_o", bufs=2, space="PSUM"))
        apool = ctx.enter_context(tc.tile_pool(name="apool", bufs=1))
        dpool = ctx.enter_context(
            tc.tile_pool(name="dpool", bufs=1, space="DRAM"))

        def q_proj(m, kmajor=False):
            pqs = [ps_pj.tile([128, 512], f32, tag="pj", name="pq")
                   for _ in range(NB)]
            order = [(k, nb) for k in range(DT) for nb in range(NB)] \
                if kmajor else [(k, nb) for nb in range(NB)
                                for k in range(DT)]
            for k, nb in order:
                nc.tensor.matmul(
                    pqs[nb], wqt[:, m, k],
                    xt[:, k, nb * 512:(nb + 1) * 512],
                    start=(k == 0), stop=(k == DT - 1))
            for nb in range(NB):
                nc.vector.tensor_scalar_add(
                    qT[:, m, nb * 512:(nb + 1) * 512], pqs[nb],
                    bqs[:, m:m + 1])

        def k_proj(m, kmajor=False):
            pks = [ps_pj.tile([128, 512], f32, tag="pj", name="pk")
                   for _ in range(NB)]
            order = [(k, nb) for k in range(DT) for nb in range(NB)] \
                if kmajor else [(k, nb) for nb in range(NB)
                                for k in range(DT)]
            for k, nb in order:
                nc.tensor.matmul(
                    pks[nb], wkt[:, m, k],
                    xt[:, k, nb * 512:(nb + 1) * 512],
                    start=(k == 0), stop=(k == DT - 1))
            for nb in range(NB):
                nc.vector.tensor_scalar_add(
                    kT[:, m, nb * 512:(nb + 1) * 512], pks[nb],
                    bks[:, m:m + 1])

        def v_proj(h0, w, sts):
            for st in sts:
                pv = ps_pj.tile([128, 512], f32, tag="pj", name="pv")
                for k in range(DT):
                    nc.tensor.matmul(
                        pv[:, :w],
                        xt[:, k, st * 128:(st + 1) * 128],
                        wvt[:, k, h0 * DK:h0 * DK + w],
                        start=(k == 0), stop=(k == DT - 1))
                nh = w // DK
                ve_h = vE[:, st, h0 * 65:(h0 + nh) * 65].rearrange(
                    "p (h e) -> p h e", e=65)
                nc.vector.tensor_add(
                    ve_h[:, :, 0:DK],
                    pv[:, :w].rearrange("p (h d) -> p h d", d=DK),
                    bvb[:, h0:h0 + nh, :])

        def oproj_pass(g, ks, first, pool=None, store_eng=None):
            mo, nb = g // NB, g % NB
            py = (pool or ps_pj).tile([128, 512], f32,
                                      tag="pj" if (pool or ps_pj) is ps_pj
                                      else "po", name="py")
            for n, k in enumerate(ks):
                nc.tensor.matmul(
                    py, wot[:, k, mo * 128:(mo + 1) * 128],
                    ao[k][:, nb * 512:(nb + 1) * 512],
                    start=(n == 0), stop=(n == len(ks) - 1))
            if first:
                nc.vector.tensor_scalar_add(ya[g], py, bos[:, mo:mo + 1])
            else:
                yt = apool.tile([128, 512], f16, tag="yt", bufs=8, name="yt")
                nc.vector.tensor_add(yt, py, ya[g])
                eng = store_eng or nc.sync
                eng.dma_start(
                    out=yT_d[mo * 128:(mo + 1) * 128,
                             nb * 512:(nb + 1) * 512],
                    in_=yt)

        def scores(m, j):
            """Emit paired-head score matmuls + exp + mask for all groups
            of (m, j). Returns [(i, lo, w, et)] for attnV."""
            out = []
            for (i, lo, mul, pat_id) in blocks[j]:
                w = 512 - lo
                pss = ps_s.tile([128, 1024], f32, tag="ps", name="pss")
                et = apool.tile([128, 1024], f16, tag="et", bufs=14,
                                name="et")
                for hh in range(HPM):
                    p0, p1 = hh * 64, hh * 64 + 64
                    nc.tensor.matmul(
                        pss[:, hh * 512 + lo:(hh + 1) * 512],
                        kT[p0:p1, m, i * 128:(i + 1) * 128],
                        qT[p0:p1, m, j * 512 + lo:(j + 1) * 512],
                        start=True, stop=True)
                # one ACT op for both heads; strided AP skips the dead
                # [512-lo, 512) gap (matmul outs can't cross PSUM banks)
                p3 = pss.rearrange("p (two q) -> p two q", two=2)
                e3 = et.rearrange("p (two q) -> p two q", two=2)
                nc.scalar.activation(out=e3[:, :, lo:512],
                                     in_=p3[:, :, lo:512], func=AF.Exp)
                if mul is not None:
                    mlo, mhi = mul
                    wm = mhi - mlo
                    m3 = mks[:, pat_off[pat_id]:pat_off[pat_id] + 2 * wm
                             ].rearrange("p (two q) -> p two q", two=2)
                    # GpSimd is nearly idle; keep the tail-critical m=5
                    # masks on the (faster) DVE
                    eng = nc.vector if m == 5 else nc.gpsimd
                    eng.tensor_mul(
                        e3[:, :, mlo:mhi], e3[:, :, mlo:mhi], m3)
                out.append((i, lo, w, et))
            return out

        def denoms(ets, rsm):
            # softmax denominators early, without waiting for attnV: M=1
            # ones-column matmuls over the exp tiles, one chain per head
            for hh in range(HPM):
                pd = ps_pj.tile([128, 512], f32, tag="pj", name="pd")
                for n, (i, lo, w, et) in enumerate(ets):
                    nc.tensor.matmul(
                        pd[0:1, lo:512], ones1,
                        et[:, hh * 512 + lo:(hh + 1) * 512],
                        start=(n == 0), stop=(n == len(ets) - 1))
                pds = apool.tile([1, 512], f16, tag="pds", bufs=2,
                                 name="pds")
                nc.vector.tensor_copy(pds, pd[0:1, :])
                nc.sync.dma_start(out=rsm[hh * 16:(hh + 1) * 16, :],
                                  in_=pds)

        def attn_v(m, j, ets, rsm, shift_eng=None):
            for hh in range(HPM):
                h = m * HPM + hh
                po = ps_o.tile([128, 512], f32, tag="po", name="po")
                for n, (i, lo, w, et) in enumerate(ets):
                    nc.tensor.matmul(
                        po[:, lo:512],
                        vE[:, i, h * 65:h * 65 + 128],
                        et[:, hh * 512 + lo:(hh + 1) * 512],
                        start=(n == 0), stop=(n == len(ets) - 1))
                stg = apool.tile([65, 512], f16, tag="stg", bufs=6,
                                 name="stg")
                if j == 0:
                    # j0 windows have ACT slack; PSUM evacuation via the
                    # scalar engine keeps the DVE queue clear
                    nc.scalar.activation(out=stg, in_=po[0:65, :],
                                         func=AF.Copy)
                else:
                    nc.vector.tensor_copy(stg, po[0:65, :])
                (shift_eng or nc.gpsimd).dma_start(
                    out=ao[m][hh * DK:(hh + 1) * DK,
                              j * 512:(j + 1) * 512],
                    in_=stg[0:DK, :])
                if rsm is not None:
                    # denominator row straight into the [32,32] reciprocal
                    # staging tile via partition-scatter DMA
                    nc.sync.dma_start(out=rsm[hh * 16:(hh + 1) * 16, :],
                                      in_=stg[DK:65, :])

        def normalize(m, j, rsm, bc1_eng=None):
            rrm = apool.tile([32, 32], f32, tag="rrm", bufs=2, name="rrm")
            nc.vector.reciprocal(rrm, rsm)
            rc16 = apool.tile([32, 32], f16, tag="rc16", bufs=2, name="rc16")
            nc.vector.tensor_copy(rc16, rrm)
            scr = dpool.tile([32, 32], f16, tag="scr", bufs=2, name="scr")
            nc.sync.dma_start(out=scr, in_=rc16)
            rt = apool.tile([128, 512], f16, tag="rt", bufs=2, name="rt")
            bc0 = bass.AP(tensor=scr.tensor, offset=scr.offset,
                          ap=[[0, DK], [1, 512]])
            bc1 = bass.AP(tensor=scr.tensor, offset=scr.offset + 512,
                          ap=[[0, DK], [1, 512]])
            nc.sync.dma_start(out=rt[0:DK, :], in_=bc0)
            (bc1_eng or nc.gpsimd).dma_start(out=rt[DK:128, :], in_=bc1)
            cols = slice(j * 512, (j + 1) * 512)
            nc.vector.tensor_mul(ao[m][:, cols], ao[m][:, cols], rt)

        # ---- main pipeline over m-tiles ----
        # Each (m, j) block: scores -> [128-mode filler PE work that hides
        # the exp window on ACT] -> attnV.  m=5 runs j=1 first so the last
        # (small) j=0 window lands next to the tail.
        q_proj(0, kmajor=True)
        k_proj(0, kmajor=True)
        for m in range(DT):
            jorder = (1, 0) if m == 5 else (0, 1)
            # both j-blocks' scores in one 64-mode burst (fewer PE tiling
            # mode switches); attnV/fills consume them in jorder
            ets_all = {}
            for j in jorder:
                ets_all[j] = scores(m, j)
            for j in jorder:
                rsm = apool.tile([32, 32], f16, tag="rsm", bufs=2,
                                 name="rsm")
                ets = ets_all[j]
                if j == 0 and m < 5:
                    if m == 0:
                        v_proj(0, 512, range(0, 4))
                    else:
                        q_proj(m + 1)
                elif j == 1 and m < 5:
                    if m == 0:
                        q_proj(1)
                        v_proj(0, 512, range(4, 8))
                        k_proj(1)
                    else:
                        k_proj(m + 1)
                    if m == 2:
                        v_proj(8, 256, range(0, 4))
                    elif m == 3:
                        v_proj(8, 256, range(4, 8))
                    elif m == 4:
                        for g in range(0, 4):
                            oproj_pass(g, range(4), True)
                elif m == 5 and j == 1:
                    for g in range(4, 10):
                        oproj_pass(g, range(4), True)
                elif m == 5 and j == 0:
                    # tail-critical block: attnV + normalize DVE ops go
                    # FIRST in the in-order DVE queue; filler (p1 rest +
                    # nb=1 output halves) after.  Last DMAs move off the
                    # gpsimd queue so its fixed end-drain overlaps compute.
                    denoms(ets, rsm)
                    attn_v(m, j, ets, None, shift_eng=nc.scalar)
                    normalize(m, j, rsm, bc1_eng=nc.scalar)
                    for g in range(10, 12):
                        oproj_pass(g, range(4), True)
                    for g in range(1, 12, 2):
                        oproj_pass(g, (4, 5), False, store_eng=nc.gpsimd)
                    continue
                attn_v(m, j, ets, None if (m, j) == (5, 0) else rsm)
                normalize(m, j, rsm)
        for n, g in enumerate(range(0, 12, 2)):
            oproj_pass(g, (4, 5), False,
                       pool=ps_o if n % 2 else ps_pj,
                       store_eng=nc.scalar if n % 2 else nc.sync)

    nc.compile()
    return nc


def prepare(x, mask, W_q, b_q, W_k, b_k, W_v, b_v, W_o, b_o):
    """Compile (cached) and build per-core input maps."""
    x = np.asarray(x, np.float32)
    mask_b = np.asarray(mask).reshape(S, S) != 0
    blocks, patterns = _classify_mask(mask_b)
    key = mask_b.tobytes()
    if key not in _CACHE:
        pat_width = sum(2 * p.shape[1] for p in patterns)
        _CACHE[key] = (_build(blocks, patterns, pat_width), patterns)
    nc, patterns = _CACHE[key]

    xT = np.ascontiguousarray(x.transpose(0, 2, 1))          # [B, D, S]

    def swz(w):
        # [D, N] -> [128, DT, N]: partition-major
        w = np.asarray(w, np.float16)
        return np.ascontiguousarray(
            w.reshape(DT, 128, w.shape[1]).transpose(1, 0, 2))

    def swz_m(w):
        # [D, D] -> [128, m, k, 128] so per-m slices are contiguous
        w = np.asarray(w, np.float16)
        return np.ascontiguousarray(
            w.reshape(DT, 128, DT, 128).transpose(1, 2, 0, 3))

    base = {
        "wq": swz_m(np.asarray(W_q, np.float32) / np.sqrt(DK)),
        "wk": swz_m(W_k),
        "wv": swz(W_v),
        "wo": swz(W_o),
        "bqko": np.ascontiguousarray(np.stack([
            (np.asarray(b_q, np.float32) / np.sqrt(DK)).reshape(DT, 128).T,
            np.asarray(b_k, np.float32).reshape(DT, 128).T,
            np.asarray(b_o, np.float32).reshape(DT, 128).T], axis=1)),
        "bv": np.ascontiguousarray(np.broadcast_to(
            np.asarray(b_v, np.float16).reshape(1, H, DK), (128, H, DK))),
    }
    if patterns:
        base["masks"] = np.ascontiguousarray(
            np.concatenate([np.concatenate([p, p], axis=1)
                            for p in patterns], axis=1))
    def xchunks(w):
        # [128, DT, S] -> [3, 128, 2, S] k-pair chunk-major (contiguous
        # per-partition runs per chunk DMA)
        return np.ascontiguousarray(
            w.reshape(128, 3, 2, S).transpose(1, 0, 2, 3))

    in_maps = [dict(base, xT=xchunks(swz(xT[c]))) for c in range(B)]
    return nc, in_maps


def kernel(**inputs):
    from concourse.bass_utils import run_bass_kernel_spmd

    nc, in_maps = prepare(**inputs)
    res = run_bass_kernel_spmd(nc, in_maps, core_ids=list(range(B)))
    out = np.stack([res.results[c]["yT"].T.astype(np.float32)
                    for c in range(B)], axis=0)
    return np.ascontiguousarray(out)


# revision 49
# speedup vs baseline: 1.1782x; 1.1782x over previous
"""Multi-head attention (B=8, S=1024, D=768, H=12) on 8 TRN2 NeuronCores.

Sharding: batch-parallel — each core computes one batch item end-to-end
(weights replicated), so no collectives are needed. Host shards x, runs the
SPMD Bass kernel on cores 0-7, gathers per-core outputs.

Per-core dataflow ([feature, seq] transposed layout, fp16 matmuls, fp32 PSUM):
  qT = (W_q/8)^T x^T          [768, 1024]  partitions 0-63 = head 2m,
  kT = W_k^T x^T              [768, 1024]  64-127 = head 2m+1 per m-tile
  v  = x W_v (+ ones col)     [1024, 768]
  scores: per (m, j-block, k-tile i): TWO K=64 matmuls (head pair) into one
    PSUM tile — auto row-tiling (tile_position (0,0)/(64,0)) runs them
    concurrently in the 64x128-tiled PE array.
  exp: single gap-free ACT op per group covering both heads (ACT is the
    attention-phase bottleneck: (N+352)/1.2ns per op).
  attnV: per head, M=65 (64 v-dims + ones row -> softmax denom), N trimmed
    causally; one [65,512] evacuation per head (Scalar-engine Copy in j0
    windows, DVE in j1), rows fanned out by DMA (partition shift for odd
    heads); masks multiply on GpSimd (DVE for the tail-critical m=5).
  normalize per (m, j-half): denominator rows partition-scatter directly
    into a [32,32] tile (all-lane reciprocal), DRAM-bounce broadcast, one
    [128,512] fp16 multiply. m=5 runs j=1 first; its j=0 denominators come
    early from M=1 ones-column matmuls so the tail chain overlaps attnV.
  yT = W_o^T out^T: pass1 (k=0..3) interleaved into m=4/5 attention stall
    windows (with the nb=1 halves of pass2), pass2 (k=4..5) after the last
    normalize; fp16 stores on the sync/scalar queues.
Every ACT exp window is filled with 128x128-mode PE work (next-m
projections, V-projection slices, output-projection passes). Host
transposes/upcasts yT back to [1024, 768] fp32.
"""

import numpy as np

B, S, D, H, DK = 8, 1024, 768, 12, 64
DT = D // 128        # 6  d-model tiles
ST = S // 128        # 8  seq tiles
NB = S // 512        # 2  512-wide seq blocks
HPM = 2              # heads per 128-row m-tile

_CACHE = {}


def _classify_mask(mask_bool):
    """mask_bool: [S, S] (q, k). Per (j, i) block descriptors for the
    scoresT layout [k, q]: list over j (512-wide q blocks) of lists of
    (i, exp_lo, mul(lo,hi)|None, pat_id|None), plus deduped mask patterns
    (each [128, w] f16, [k, q], stored duplicated side by side so one DVE
    op can mask both heads' copies)."""
    patterns = []
    pat_index = {}
    blocks = []
    for j in range(NB):
        row = []
        for i in range(ST):
            sub = mask_bool[j * 512:(j + 1) * 512, i * 128:(i + 1) * 128].T
            # sub: [k 128, q 512]
            if not sub.any():
                continue
            col_any = sub.any(axis=0)
            col_all = sub.all(axis=0)
            exp_lo = int(np.argmax(col_any))
            assert not col_any[:exp_lo].any()
            mixed = ~col_all
            mixed[:exp_lo] = False
            mul = None
            pat_id = None
            if mixed.any():
                lo = int(np.argmax(mixed))
                hi = int(len(mixed) - np.argmax(mixed[::-1]))
                assert col_all[hi:].all() and col_all[exp_lo:lo].all()
                pat = sub[:, lo:hi].astype(np.float16)
                key = (pat.shape[1], pat.tobytes())
                if key not in pat_index:
                    pat_index[key] = len(patterns)
                    patterns.append(pat)
                mul = (lo, hi)
                pat_id = pat_index[key]
            row.append((i, exp_lo, mul, pat_id))
        blocks.append(row)
    return blocks, patterns


def _build(blocks, patterns, pat_width):
    import concourse.bass as bass
    import concourse.bacc as bacc
    import concourse.mybir as mybir
    import concourse.tile as tile
    from contextlib import ExitStack

    f32 = mybir.dt.float32
    f16 = mybir.dt.float16
    AF = mybir.ActivationFunctionType

    nc = bacc.Bacc("TRN2", target_bir_lowering=False, debug=False)

    xT_d = nc.dram_tensor("xT", [3, 128, 2, S], f16,
                          kind="ExternalInput").ap()   # k-pair chunk major
    wq_d = nc.dram_tensor("wq", [128, DT, DT, 128], f16,
                          kind="ExternalInput").ap()   # [p, m, k, c]
    wk_d = nc.dram_tensor("wk", [128, DT, DT, 128], f16,
                          kind="ExternalInput").ap()
    wv_d = nc.dram_tensor("wv", [128, DT, D], f16, kind="ExternalInput").ap()
    wo_d = nc.dram_tensor("wo", [128, DT, D], f16, kind="ExternalInput").ap()
    bqko_d = nc.dram_tensor("bqko", [128, 3, DT], f32,
                            kind="ExternalInput").ap()
    bv_d = nc.dram_tensor("bv", [128, H, DK], f16,
                          kind="ExternalInput").ap()
    yT_d = nc.dram_tensor("yT", [D, S], f16, kind="ExternalOutput").ap()
    if pat_width:
        mk_d = nc.dram_tensor("masks", [128, pat_width], f16,
                              kind="ExternalInput").ap()

    pat_off = []
    off = 0
    for p in patterns:
        pat_off.append(off)
        off += 2 * p.shape[1]

    with tile.TileContext(nc) as tc, ExitStack() as ctx:
        cpool = ctx.enter_context(tc.tile_pool(name="cpool", bufs=1))
        qT = cpool.tile([128, DT, S], f16)
        kT = cpool.tile([128, DT, S], f16)
        vE = cpool.tile([128, ST, H * 65 + 63], f16)
        ao = [cpool.tile([128, S], f16, name=f"ao{m}") for m in range(DT)]
        ya = [cpool.tile([128, 512], f16, name=f"ya{g}") for g in range(12)]
        bvb = cpool.tile([128, H, DK], f16)
        ones1 = cpool.tile([128, 1], f16)
        bqko = cpool.tile([128, 3, DT], f32)
        bqs = bqko[:, 0]
        bks = bqko[:, 1]
        bos = bqko[:, 2]

        xt = cpool.tile([128, DT, S], f16)
        wvt = cpool.tile([128, DT, D], f16)
        wqt = cpool.tile([128, DT, DT, 128], f16)
        wkt = cpool.tile([128, DT, DT, 128], f16)
        wot = cpool.tile([128, DT, D], f16)

        # staged input loads. x is split across three otherwise-idle DMA
        # queues in parallel (per-queue DMA bw is only ~120 GB/s); the
        # sync queue carries just the weights; small constants ride the
        # scalar queue. Chains keep the SDMA round-robin from starving
        # the early pieces.
        mks = cpool.tile([128, pat_width], f16, name="mks") \
            if pat_width else None
        d_x0 = nc.gpsimd.dma_start(out=xt[:, 0:2], in_=xT_d[0])
        d_wva = nc.gpsimd.dma_start(out=wvt[:, :, 0:512],
                                    in_=wv_d[:, :, 0:512])
        d_wvb = nc.gpsimd.dma_start(out=wvt[:, :, 512:768],
                                    in_=wv_d[:, :, 512:768])
        d_wo = nc.gpsimd.dma_start(out=wot, in_=wo_d)
        gchain = [d_x0, d_wva, d_wvb, d_wo]
        d_x1 = nc.scalar.dma_start(out=xt[:, 2:4], in_=xT_d[1])
        cchain = [d_x1, nc.scalar.dma_start(out=bqko, in_=bqko_d)]
        if pat_width:
            cchain.append(nc.scalar.dma_start(out=mks, in_=mk_d))
        cchain.append(nc.scalar.dma_start(out=bvb, in_=bv_d))
        d_wq0 = nc.sync.dma_start(out=wqt[:, 0], in_=wq_d[:, 0])
        d_x2 = nc.sync.dma_start(out=xt[:, 4:6], in_=xT_d[2])
        d_wk0 = nc.sync.dma_start(out=wkt[:, 0], in_=wk_d[:, 0])
        schain = [d_wq0, d_x2, d_wk0,
                  nc.sync.dma_start(out=wqt[:, 1:6], in_=wq_d[:, 1:6]),
                  nc.sync.dma_start(out=wkt[:, 1:6], in_=wk_d[:, 1:6])]
        for chain in (gchain, schain, cchain):
            for a, b in zip(chain[1:], chain):
                tile.add_dep_helper(a.ins, b.ins, reason="stage inputs")

        # ones columns for the softmax-denominator rows; zero the tail so
        # the last heads' 128-col lhsT windows read defined data
        for st in range(ST):
            ve_h = vE[:, st, 0:H * 65].rearrange("p (h e) -> p h e", e=65)
            nc.vector.memset(ve_h[:, :, DK:DK + 1], 1.0)
        nc.vector.memset(vE[:, :, H * 65:], 0.0)
        nc.vector.memset(ones1, 1.0)

        ps_pj = ctx.enter_context(
            tc.tile_pool(name="ps_pj", bufs=2, space="PSUM"))
        ps_s = ctx.enter_context(
            tc.tile_pool(name="ps_s", bufs=2, space="PSUM"))
        ps_o = ctx.enter_context(
            tc.tile_pool(name="ps_o", bufs=2, space="PSUM"))
        apool = ctx.enter_context(tc.tile_pool(name="apool", bufs=1))
        dpool = ctx.enter_context(
            tc.tile_pool(name="dpool", bufs=1, space="DRAM"))

        def q_proj(m, kmajor=False):
            pqs = [ps_pj.tile([128, 512], f32, tag="pj", name="pq")
                   for _ in range(NB)]
            order = [(k, nb) for k in range(DT) for nb in range(NB)] \
                if kmajor else [(k, nb) for nb in range(NB)
                                for k in range(DT)]
            for k, nb in order:
                nc.tensor.matmul(
                    pqs[nb], wqt[:, m, k],
                    xt[:, k, nb * 512:(nb + 1) * 512],
                    start=(k == 0), stop=(k == DT - 1))
            for nb in range(NB):
                nc.vector.tensor_scalar_add(
                    qT[:, m, nb * 512:(nb + 1) * 512], pqs[nb],
                    bqs[:, m:m + 1])

        def k_proj(m, kmajor=False):
            pks = [ps_pj.tile([128, 512], f32, tag="pj", name="pk")
                   for _ in range(NB)]
            order = [(k, nb) for k in range(DT) for nb in range(NB)] \
                if kmajor else [(k, nb) for nb in range(NB)
                                for k in range(DT)]
            for k, nb in order:
                nc.tensor.matmul(
                    pks[nb], wkt[:, m, k],
                    xt[:, k, nb * 512:(nb + 1) * 512],
                    start=(k == 0), stop=(k == DT - 1))
            for nb in range(NB):
                nc.vector.tensor_scalar_add(
                    kT[:, m, nb * 512:(nb + 1) * 512], pks[nb],
                    bks[:, m:m + 1])

        def v_proj(h0, w, sts):
            for st in sts:
                pv = ps_pj.tile([128, 512], f32, tag="pj", name="pv")
                for k in range(DT):
                    nc.tensor.matmul(
                        pv[:, :w],
                        xt[:, k, st * 128:(st + 1) * 128],
                        wvt[:, k, h0 * DK:h0 * DK + w],
                        start=(k == 0), stop=(k == DT - 1))
                nh = w // DK
                ve_h = vE[:, st, h0 * 65:(h0 + nh) * 65].rearrange(
                    "p (h e) -> p h e", e=65)
                nc.vector.tensor_add(
                    ve_h[:, :, 0:DK],
                    pv[:, :w].rearrange("p (h d) -> p h d", d=DK),
                    bvb[:, h0:h0 + nh, :])

        def oproj_pass(g, ks, first, pool=None, store_eng=None):
            mo, nb = g // NB, g % NB
            py = (pool or ps_pj).tile([128, 512], f32,
                                      tag="pj" if (pool or ps_pj) is ps_pj
                                      else "po", name="py")
            for n, k in enumerate(ks):
                nc.tensor.matmul(
                    py, wot[:, k, mo * 128:(mo + 1) * 128],
                    ao[k][:, nb * 512:(nb + 1) * 512],
                    start=(n == 0), stop=(n == len(ks) - 1))
            if first:
                nc.vector.tensor_scalar_add(ya[g], py, bos[:, mo:mo + 1])
            else:
                yt = apool.tile([128, 512], f16, tag="yt", bufs=8, name="yt")
                nc.vector.tensor_add(yt, py, ya[g])
                eng = store_eng or nc.sync
                eng.dma_start(
                    out=yT_d[mo * 128:(mo + 1) * 128,
                             nb * 512:(nb + 1) * 512],
                    in_=yt)

        def scores(m, j):
            """Emit paired-head score matmuls + exp + mask for all groups
            of (m, j). Returns [(i, lo, w, et)] for attnV."""
            out = []
            for (i, lo, mul, pat_id) in blocks[j]:
                w = 512 - lo
                pss = ps_s.tile([128, 1024], f32, tag="ps", name="pss")
                et = apool.tile([128, 1024], f16, tag="et", bufs=14,
                                name="et")
                for hh in range(HPM):
                    p0, p1 = hh * 64, hh * 64 + 64
                    nc.tensor.matmul(
                        pss[:, hh * 512 + lo:(hh + 1) * 512],
                        kT[p0:p1, m, i * 128:(i + 1) * 128],
                        qT[p0:p1, m, j * 512 + lo:(j + 1) * 512],
                        start=True, stop=True)
                # one ACT op for both heads; strided AP skips the dead
                # [512-lo, 512) gap (matmul outs can't cross PSUM banks)
                p3 = pss.rearrange("p (two q) -> p two q", two=2)
                e3 = et.rearrange("p (two q) -> p two q", two=2)
                nc.scalar.activation(out=e3[:, :, lo:512],
                                     in_=p3[:, :, lo:512], func=AF.Exp)
                if mul is not None:
                    mlo, mhi = mul
                    wm = mhi - mlo
                    m3 = mks[:, pat_off[pat_id]:pat_off[pat_id] + 2 * wm
                             ].rearrange("p (two q) -> p two q", two=2)
                    # GpSimd is nearly idle; keep the tail-critical m=5
                    # masks on the (faster) DVE
                    eng = nc.vector if m == 5 else nc.gpsimd
                    eng.tensor_mul(
                        e3[:, :, mlo:mhi], e3[:, :, mlo:mhi], m3)
                out.append((i, lo, w, et))
            return out

        def denoms(ets, rsm):
            # softmax denominators early, without waiting for attnV: M=1
            # ones-column matmuls over the exp tiles, one chain per head
            for hh in range(HPM):
                pd = ps_pj.tile([128, 512], f32, tag="pj", name="pd")
                for n, (i, lo, w, et) in enumerate(ets):
                    nc.tensor.matmul(
                        pd[0:1, lo:512], ones1,
                        et[:, hh * 512 + lo:(hh + 1) * 512],
                        start=(n == 0), stop=(n == len(ets) - 1))
                pds = apool.tile([1, 512], f16, tag="pds", bufs=2,
                                 name="pds")
                nc.vector.tensor_copy(pds, pd[0:1, :])
                nc.sync.dma_start(out=rsm[hh * 16:(hh + 1) * 16, :],
                                  in_=pds)

        def attn_v(m, j, ets, rsm, shift_eng=None):
            for hh in range(HPM):
                h = m * HPM + hh
                po = ps_o.tile([128, 512], f32, tag="po", name="po")
                for n, (i, lo, w, et) in enumerate(ets):
                    nc.tensor.matmul(
                        po[:, lo:512],
                        vE[:, i, h * 65:h * 65 + 128],
                        et[:, hh * 512 + lo:(hh + 1) * 512],
                        start=(n == 0), stop=(n == len(ets) - 1))
                stg = apool.tile([65, 512], f16, tag="stg", bufs=6,
                                 name="stg")
                if j == 0:
                    # j0 windows have ACT slack; PSUM evacuation via the
                    # scalar engine keeps the DVE queue clear
                    nc.scalar.activation(out=stg, in_=po[0:65, :],
                                         func=AF.Copy)
                else:
                    nc.vector.tensor_copy(stg, po[0:65, :])
                (shift_eng or nc.gpsimd).dma_start(
                    out=ao[m][hh * DK:(hh + 1) * DK,
                              j * 512:(j + 1) * 512],
                    in_=stg[0:DK, :])
                if rsm is not None:
                    # denominator row straight into the [32,32] reciprocal
                    # staging tile via partition-scatter DMA
                    nc.sync.dma_start(out=rsm[hh * 16:(hh + 1) * 16, :],
                                      in_=stg[DK:65, :])

        def normalize(m, j, rsm, bc1_eng=None):
            rrm = apool.tile([32, 32], f32, tag="rrm", bufs=2, name="rrm")
            nc.vector.reciprocal(rrm, rsm)
            rc16 = apool.tile([32, 32], f16, tag="rc16", bufs=2, name="rc16")
            nc.vector.tensor_copy(rc16, rrm)
            scr = dpool.tile([32, 32], f16, tag="scr", bufs=2, name="scr")
            nc.sync.dma_start(out=scr, in_=rc16)
            rt = apool.tile([128, 512], f16, tag="rt", bufs=2, name="rt")
            bc0 = bass.AP(tensor=scr.tensor, offset=scr.offset,
                          ap=[[0, DK], [1, 512]])
            bc1 = bass.AP(tensor=scr.tensor, offset=scr.offset + 512,
                          ap=[[0, DK], [1, 512]])
            nc.sync.dma_start(out=rt[0:DK, :], in_=bc0)
            (bc1_eng or nc.gpsimd).dma_start(out=rt[DK:128, :], in_=bc1)
            cols = slice(j * 512, (j + 1) * 512)
            nc.vector.tensor_mul(ao[m][:, cols], ao[m][:, cols], rt)

        # ---- main pipeline over m-tiles ----
        # Each (m, j) block: scores -> [128-mode filler PE work that hides
        # the exp window on ACT] -> attnV.  m=5 runs j=1 first so the last
        # (small) j=0 window lands next to the tail.
        q_proj(0, kmajor=True)
        k_proj(0, kmajor=True)
        for m in range(DT):
            jorder = (1, 0) if m == 5 else (0, 1)
            # both j-blocks' scores in one 64-mode burst (fewer PE tiling
            # mode switches); attnV/fills consume them in jorder
            ets_all = {}
            for j in jorder:
                ets_all[j] = scores(m, j)
            for j in jorder:
                rsm = apool.tile([32, 32], f16, tag="rsm", bufs=2,
                                 name="rsm")
                ets = ets_all[j]
                if j == 0 and m < 5:
                    if m == 0:
                        v_proj(0, 512, range(0, 4))
                    else:
                        q_proj(m + 1)
                elif j == 1 and m < 5:
                    if m == 0:
                        q_proj(1)
                        v_proj(0, 512, range(4, 8))
                        k_proj(1)
                    else:
                        k_proj(m + 1)
                    if m == 2:
                        v_proj(8, 256, range(0, 4))
                    elif m == 3:
                        v_proj(8, 256, range(4, 8))
                    elif m == 4:
                        for g in range(0, 4):
                            oproj_pass(g, range(4), True)
                elif m == 5 and j == 1:
                    for g in range(4, 10):
                        oproj_pass(g, range(4), True)
                elif m == 5 and j == 0:
                    # tail-critical block: attnV + normalize DVE ops go
                    # FIRST in the in-order DVE queue; filler (p1 rest +
                    # nb=1 output halves) after.  Last DMAs move off the
                    # gpsimd queue so its fixed end-drain overlaps compute.
                    denoms(ets, rsm)
                    attn_v(m, j, ets, None, shift_eng=nc.scalar)
                    normalize(m, j, rsm, bc1_eng=nc.scalar)
                    for g in range(10, 12):
                        oproj_pass(g, range(4), True)
                    for g in range(1, 12, 2):
                        oproj_pass(g, (4, 5), False, store_eng=nc.gpsimd)
                    continue
                attn_v(m, j, ets, None if (m, j) == (5, 0) else rsm)
                normalize(m, j, rsm)
        for n, g in enumerate(range(0, 12, 2)):
            oproj_pass(g, (4, 5), False,
                       pool=ps_o if n % 2 else ps_pj,
                       store_eng=nc.scalar if n % 2 else nc.sync)

    nc.compile()
    return nc


def prepare(x, mask, W_q, b_q, W_k, b_k, W_v, b_v, W_o, b_o):
    """Compile (cached) and build per-core input maps."""
    x = np.asarray(x, np.float32)
    mask_b = np.asarray(mask).reshape(S, S) != 0
    blocks, patterns = _classify_mask(mask_b)
    key = mask_b.tobytes()
    if key not in _CACHE:
        pat_width = sum(2 * p.shape[1] for p in patterns)
        _CACHE[key] = (_build(blocks, patterns, pat_width), patterns)
    nc, patterns = _CACHE[key]

    xT = np.ascontiguousarray(x.transpose(0, 2, 1))          # [B, D, S]

    def swz(w):
        # [D, N] -> [128, DT, N]: partition-major
        w = np.asarray(w, np.float16)
        return np.ascontiguousarray(
            w.reshape(DT, 128, w.shape[1]).transpose(1, 0, 2))

    def swz_m(w):
        # [D, D] -> [128, m, k, 128] so per-m slices are contiguous
        w = np.asarray(w, np.float16)
        return np.ascontiguousarray(
            w.reshape(DT, 128, DT, 128).transpose(1, 2, 0, 3))

    base = {
        "wq": swz_m(np.asarray(W_q, np.float32) / np.sqrt(DK)),
        "wk": swz_m(W_k),
        "wv": swz(W_v),
        "wo": swz(W_o),
        "bqko": np.ascontiguousarray(np.stack([
            (np.asarray(b_q, np.float32) / np.sqrt(DK)).reshape(DT, 128).T,
            np.asarray(b_k, np.float32).reshape(DT, 128).T,
            np.asarray(b_o, np.float32).reshape(DT, 128).T], axis=1)),
        "bv": np.ascontiguousarray(np.broadcast_to(
            np.asarray(b_v, np.float16).reshape(1, H, DK), (128, H, DK))),
    }
    if patterns:
        base["masks"] = np.ascontiguousarray(
            np.concatenate([np.concatenate([p, p], axis=1)
                            for p in patterns], axis=1))
    def xchunks(w):
        # [128, DT, S] -> [3, 128, 2, S] k-pair chunk-major (contiguous
        # per-partition runs per chunk DMA)
        return np.ascontiguousarray(
            w.reshape(128, 3, 2, S).transpose(1, 0, 2, 3))

    in_maps = [dict(base, xT=xchunks(swz(xT[c]))) for c in range(B)]
    return nc, in_maps


def kernel(**inputs):
    from concourse.bass_utils import run_bass_kernel_spmd

    nc, in_maps = prepare(**inputs)
    res = run_bass_kernel_spmd(nc, in_maps, core_ids=list(range(B)))
    out = np.stack([res.results[c]["yT"].T.astype(np.float32)
                    for c in range(B)], axis=0)
    return np.ascontiguousarray(out)
